# revision 10
# baseline (speedup 1.0000x reference)
"""Block-sparse (DeepSpeed fixed-layout) self-attention on 8 Trainium2 cores.

Strategy
--------
Shard the 32 (batch, head) slices across 8 cores (4 each, pure data parallel).
For each slice, queries are processed in windows of 128 rows (4 key-blocks of
32). The union of active key blocks for a window is split into "chunk slots"
of up to 128 keys; slots are deduplicated across windows (the fixed layout's
global stripe makes most slots shared). Host-side numpy pre-gathers:
  QT  [S, 64, L]            query transposed (hd on partitions)
  KTg [S, 64, nch*128]      gathered+transposed key chunks
  Vg  [S, 128, nch, 65]     gathered value chunks with a ones column
On device, per window and chunk:
  S_T  = KT_chunk.T-matmul (scores arrive keys-on-partitions: no P transpose)
  P    = exp(scale * S_T)   on ACT, straight from PSUM
  mask: memset invalid (key-block, query-block) cells to zero
  O~  += P.T @ [V | 1]      accumulated in PSUM; col 64 = softmax denominator
then O = O~[:, :64] * (1 / O~[:, 64]) and DMA out. exp() needs no max
subtraction: scores are ~N(0,1) after scaling, far from fp32 overflow.
"""

import sys

sys.path.insert(0, "/opt/trn_rl_repo")

import numpy as np

N_CORES = 8

# dtype knobs: storage+matmul dtype for scores (QT/KTg) and probs (P/Vg)
S_DT_NAME = "float16"
P_DT_NAME = "float16"

_cache = {}


def _build_plan(rows, cols, nb, qw):
    """Per query-window chunk lists + deduplicated key-chunk slots.

    Returns (windows, slot_blocks):
      windows: list (one per window) of chunks (slot_id, n_blocks, valid)
               where valid[kb, j] says whether key-block kb of the chunk is
               attended by query-block j of the window.
      slot_blocks: slot_id -> list of key block ids stored in that slot.
    """
    from collections import Counter

    row_cnt = [Counter() for _ in range(nb)]
    for r, c in zip(rows.tolist(), cols.tolist()):
        row_cnt[int(r)][int(c)] += 1

    slots = {}
    slot_blocks = []
    windows = []
    for w0 in range(0, nb, qw):
        cnts = [row_cnt[w0 + j] for j in range(qw)]
        cols_set = sorted(set().union(*[set(c.keys()) for c in cnts]))
        entries = []  # (block, occurrence)
        for c in cols_set:
            m = max(cnt[c] for cnt in cnts)
            entries.extend((c, k) for k in range(m))
        # maximal equal-stride runs -> chunk boundaries shared across windows
        runs = []
        i, n = 0, len(entries)
        while i < n:
            if i + 1 < n:
                stride = entries[i + 1][0] - entries[i][0]
                j = i + 1
                while j + 1 < n and entries[j + 1][0] - entries[j][0] == stride:
                    j += 1
            else:
                stride, j = 1, i
            runs.append((i, j + 1, stride))
            i = j + 1
        wchunks = []
        for a, b, stride in runs:
            for t in range(a, b, qw):
                grp = entries[t : min(t + qw, b)]
                start = grp[0][0]
                key = (start, stride if len(grp) > 1 else 1)
                sid = slots.get(key)
                if sid is None:
                    sid = len(slot_blocks)
                    slots[key] = sid
                    slot_blocks.append([])
                if len(slot_blocks[sid]) < len(grp):
                    slot_blocks[sid] = [start + key[1] * u for u in range(len(grp))]
                valid = np.ones((len(grp), qw), dtype=bool)
                for kb, (c, k) in enumerate(grp):
                    for j in range(qw):
                        valid[kb, j] = k < cnts[j][c]
                wchunks.append((sid, len(grp), valid))
        windows.append(wchunks)
    return windows, slot_blocks


def _zero_regions(valid, bs):
    """Invalid (key-block, query-block) cells as memset rectangles."""
    regs = []
    nkb, qw = valid.shape
    for kb in range(nkb):
        j = 0
        while j < qw:
            if not valid[kb, j]:
                j0 = j
                while j < qw and not valid[kb, j]:
                    j += 1
                regs.append((kb * bs, (kb + 1) * bs, j0 * bs, j * bs))
            else:
                j += 1
    return regs


def _build_nc(windows, slot_blocks, dims, s_dt_name, p_dt_name, repeat):
    import os

    import concourse.mybir as mybir
    import concourse.tile as tile
    from concourse import bacc

    ablate = set(os.environ.get("KERNEL_ABLATE", "").split(","))
    stp_bufs = int(os.environ.get("TUNE_STP", "4"))
    ptp_bufs = int(os.environ.get("TUNE_PTP", "6"))
    ovp_bufs = int(os.environ.get("TUNE_OVP", "2"))

    S, L, HD, bs, qw, nch = dims
    s_dt = getattr(mybir.dt, s_dt_name)
    p_dt = getattr(mybir.dt, p_dt_name)
    f32 = mybir.dt.float32
    nq = qw * bs
    scale = float(HD) ** -0.5

    nc = bacc.Bacc("TRN2", debug=False)
    # dummy repeat-sized input: makes each repeat-variant's HLO structurally
    # unique so the neuron compile cache cannot alias them
    rtag_d = nc.dram_tensor("rtag", [1, 16 * repeat], mybir.dt.float32,
                            kind="ExternalInput")
    qt_d = nc.dram_tensor("qt", [S, HD, L], s_dt, kind="ExternalInput")
    ktg_d = nc.dram_tensor("ktg", [S, HD, nch * 128], s_dt, kind="ExternalInput")
    vg_d = nc.dram_tensor("vg", [S, 128, nch, HD + 1], p_dt, kind="ExternalInput")
    out_d = nc.dram_tensor("out", [S, L, HD], f32, kind="ExternalOutput")

    with tile.TileContext(nc) as tc:
        with (
            tc.tile_pool(name="big", bufs=2) as big,
            tc.tile_pool(name="ptp", bufs=ptp_bufs) as ptp,
            tc.tile_pool(name="onp", bufs=4) as onp,
            tc.tile_pool(name="stp", bufs=stp_bufs, space="PSUM") as stp,
            tc.tile_pool(name="ovp", bufs=ovp_bufs, space="PSUM") as ovp,
        ):
            rtag_t = big.tile([1, 16 * repeat], mybir.dt.float32, tag="rtag")
            nc.sync.dma_start(out=rtag_t, in_=rtag_d.ap())
            for _rep in range(repeat):
                for s in range(S):
                    qt_t = big.tile([HD, L], s_dt, tag="qt")
                    nc.sync.dma_start(out=qt_t, in_=qt_d.ap()[s])
                    ktg_t = big.tile([HD, nch * 128], s_dt, tag="ktg")
                    nc.sync.dma_start(out=ktg_t, in_=ktg_d.ap()[s])
                    vg_t = big.tile([128, nch, HD + 1], p_dt, tag="vg")
                    nc.sync.dma_start(out=vg_t, in_=vg_d.ap()[s])
                    for wi, wchunks in enumerate(windows):
                        ov_t = ovp.tile([128, HD + 1], f32, tag="ov")
                        nchunks = len(wchunks)
                        # phase 1: all score matmuls + exp for this window,
                        # so PE never stalls waiting on a just-computed exp
                        pts = []
                        for ci, (sid, nblk, valid) in enumerate(wchunks):
                            nk = nblk * bs
                            st_t = stp.tile([128, nq], f32, tag="st")
                            nc.tensor.matmul(
                                st_t[:nk, :],
                                lhsT=ktg_t[:, sid * 128 : sid * 128 + nk],
                                rhs=qt_t[:, wi * nq : (wi + 1) * nq],
                                start=True,
                                stop=True,
                            )
                            pt_t = ptp.tile([128, nq], p_dt, tag="pt")
                            if "noexp" in ablate:
                                nc.vector.tensor_copy(pt_t[:nk, :], st_t[:nk, :])
                            else:
                                nc.scalar.activation(
                                    pt_t[:nk, :],
                                    st_t[:nk, :],
                                    mybir.ActivationFunctionType.Exp,
                                    scale=scale,
                                )
                            if "nomask" not in ablate:
                                for p0, p1, q0, q1 in _zero_regions(valid, bs):
                                    nc.vector.memset(pt_t[p0:p1, q0:q1], 0.0)
                            pts.append((pt_t, sid, nk))
                        # phase 2: accumulate P @ [V|1]
                        if "nopv" not in ablate:
                            for ci, (pt_t, sid, nk) in enumerate(pts):
                                nc.tensor.matmul(
                                    ov_t,
                                    lhsT=pt_t[:nk, :],
                                    rhs=vg_t[:nk, sid, :],
                                    start=(ci == 0),
                                    stop=(ci == nchunks - 1),
                                )
                        o_t = onp.tile([128, HD], f32, tag="o")
                        if "nonorm" in ablate:
                            nc.vector.tensor_copy(o_t, ov_t[:, 0:HD])
                        else:
                            rec_t = onp.tile([128, 1], f32, tag="rec")
                            nc.vector.reciprocal(rec_t, ov_t[:, HD : HD + 1])
                            o_t2 = o_t
                            nc.vector.tensor_scalar_mul(o_t2, ov_t[:, 0:HD], rec_t)
                        nc.sync.dma_start(
                            out=out_d.ap()[s, wi * nq : (wi + 1) * nq, :], in_=o_t
                        )
    nc.compile()
    return nc


def _np_dt(name):
    if name == "float32":
        return np.float32
    if name == "float16":
        return np.float16
    import ml_dtypes

    return np.dtype(getattr(ml_dtypes, name))


def _prepare(query, key, value, rows, cols, block, repeat):
    B, H, L, HD = query.shape
    bs = int(block)
    nb = L // bs
    qw = max(1, 128 // bs)
    cache_key = (
        query.shape,
        bs,
        rows.tobytes(),
        cols.tobytes(),
        S_DT_NAME,
        P_DT_NAME,
        repeat,
    )
    if cache_key in _cache:
        return _cache[cache_key]

    windows, slot_blocks = _build_plan(np.asarray(rows), np.asarray(cols), nb, qw)
    nch = len(slot_blocks)
    dims = (B * H // N_CORES, L, HD, bs, qw, nch)
    nc = _build_nc(windows, slot_blocks, dims, S_DT_NAME, P_DT_NAME, repeat)
    _cache[cache_key] = (nc, windows, slot_blocks, dims)
    return _cache[cache_key]


def kernel(query, key, value, rows, cols, block):
    from concourse import bass_utils

    query = np.asarray(query)
    key = np.asarray(key)
    value = np.asarray(value)
    rows = np.asarray(rows)
    cols = np.asarray(cols)

    nc, windows, slot_blocks, dims = _prepare(
        query, key, value, rows, cols, block, repeat=1
    )
    S, L, HD, bs, qw, nch = dims
    B, H = query.shape[0], query.shape[1]
    BH = B * H
    s_np = _np_dt(S_DT_NAME)
    p_np = _np_dt(P_DT_NAME)

    q2 = query.reshape(BH, L, HD)
    k2 = key.reshape(BH, L, HD)
    v2 = value.reshape(BH, L, HD)
    qt = np.ascontiguousarray(q2.transpose(0, 2, 1)).astype(s_np)
    ktg = np.zeros((BH, HD, nch, 128), s_np)
    vg = np.zeros((BH, 128, nch, HD + 1), p_np)
    for sid, blocks in enumerate(slot_blocks):
        for kb, c in enumerate(blocks):
            kblk = k2[:, c * bs : (c + 1) * bs, :]
            ktg[:, :, sid, kb * bs : (kb + 1) * bs] = kblk.transpose(0, 2, 1)
            vg[:, kb * bs : (kb + 1) * bs, sid, :HD] = v2[:, c * bs : (c + 1) * bs, :]
            vg[:, kb * bs : (kb + 1) * bs, sid, HD] = 1.0
    ktg = ktg.reshape(BH, HD, nch * 128)

    rtag = np.zeros((1, 16), np.float32)
    in_maps = []
    for c in range(N_CORES):
        sl = slice(c * S, (c + 1) * S)
        in_maps.append({"qt": qt[sl], "ktg": ktg[sl], "vg": vg[sl], "rtag": rtag})

    res = bass_utils.run_bass_kernel_spmd(nc, in_maps, core_ids=list(range(N_CORES)))
    out = np.stack([res.results[c]["out"] for c in range(N_CORES)])  # [8, S, L, HD]
    return out.reshape(B, H, L, HD).astype(np.float32)


# revision 19
# speedup vs baseline: 1.3561x; 1.3561x over previous
"""Block-sparse (DeepSpeed fixed-layout) self-attention on 8 Trainium2 cores.

Strategy
--------
Shard the 32 (batch, head) slices across 8 cores (4 each, pure data parallel).
For each slice, queries are processed in windows of 128 rows (4 key-blocks of
32). The union of active key blocks for a window is split into "chunk slots"
of up to 128 keys; slots are deduplicated across windows (the fixed layout's
global stripe makes most slots shared). Host-side numpy pre-gathers:
  QT  [S, 64, L]            query transposed (hd on partitions)
  KTg [S, 64, nch*128]      gathered+transposed key chunks
  Vg  [S, 128, nch, 65]     gathered value chunks with a ones column
On device, per window and chunk:
  S_T  = KT_chunk.T-matmul (scores arrive keys-on-partitions: no P transpose)
  P    = exp(scale * S_T)   on ACT, straight from PSUM
  mask: memset invalid (key-block, query-block) cells to zero
  O~  += P.T @ [V | 1]      accumulated in PSUM; col 64 = softmax denominator
then O = O~[:, :64] * (1 / O~[:, 64]) and DMA out. exp() needs no max
subtraction: scores are ~N(0,1) after scaling, far from fp32 overflow.
"""

import sys

sys.path.insert(0, "/opt/trn_rl_repo")

import numpy as np

N_CORES = 8
_KVER = "v8"  # bump on any codegen change: feeds the rtag config fingerprint

# dtype knobs: storage+matmul dtype for scores (QT/KTg) and probs (P/Vg)
S_DT_NAME = "float16"
P_DT_NAME = "float16"

_cache = {}


def _build_plan(rows, cols, nb, qw):
    """Per query-window chunk lists + deduplicated key-chunk slots.

    Returns (windows, slot_blocks):
      windows: list (one per window) of chunks (slot_id, n_blocks, valid)
               where valid[kb, j] says whether key-block kb of the chunk is
               attended by query-block j of the window.
      slot_blocks: slot_id -> list of key block ids stored in that slot.
    """
    from collections import Counter

    row_cnt = [Counter() for _ in range(nb)]
    for r, c in zip(rows.tolist(), cols.tolist()):
        row_cnt[int(r)][int(c)] += 1

    slots = {}
    slot_blocks = []
    windows = []
    for w0 in range(0, nb, qw):
        cnts = [row_cnt[w0 + j] for j in range(qw)]
        cols_set = sorted(set().union(*[set(c.keys()) for c in cnts]))
        entries = []  # (block, occurrence)
        for c in cols_set:
            m = max(cnt[c] for cnt in cnts)
            entries.extend((c, k) for k in range(m))
        # maximal equal-stride runs -> chunk boundaries shared across windows
        runs = []
        i, n = 0, len(entries)
        while i < n:
            if i + 1 < n:
                stride = entries[i + 1][0] - entries[i][0]
                j = i + 1
                while j + 1 < n and entries[j + 1][0] - entries[j][0] == stride:
                    j += 1
            else:
                stride, j = 1, i
            runs.append((i, j + 1, stride))
            i = j + 1
        wchunks = []
        for a, b, stride in runs:
            for t in range(a, b, qw):
                grp = entries[t : min(t + qw, b)]
                start = grp[0][0]
                key = (start, stride if len(grp) > 1 else 1)
                sid = slots.get(key)
                if sid is None:
                    sid = len(slot_blocks)
                    slots[key] = sid
                    slot_blocks.append([])
                if len(slot_blocks[sid]) < len(grp):
                    slot_blocks[sid] = [start + key[1] * u for u in range(len(grp))]
                valid = np.ones((len(grp), qw), dtype=bool)
                for kb, (c, k) in enumerate(grp):
                    for j in range(qw):
                        valid[kb, j] = k < cnts[j][c]
                wchunks.append((sid, len(grp), valid))
        # does any query row of this window have no valid key at all?
        anyvalid = np.zeros(qw, dtype=bool)
        for _sid, _n, valid in wchunks:
            anyvalid |= valid.any(axis=0)
        windows.append((wchunks, not anyvalid.all()))
    return windows, slot_blocks


def _zero_regions(valid, bs):
    """Invalid (key-block, query-block) cells as memset rectangles."""
    regs = []
    nkb, qw = valid.shape
    for kb in range(nkb):
        j = 0
        while j < qw:
            if not valid[kb, j]:
                j0 = j
                while j < qw and not valid[kb, j]:
                    j += 1
                regs.append((kb * bs, (kb + 1) * bs, j0 * bs, j * bs))
            else:
                j += 1
    return regs


def _group_plan(windows, qw_groups=4):
    """Group consecutive windows; extract chunks shared (all-valid) by every
    window of the group so their S_T/exp run once at group width."""
    groups = []
    for g0 in range(0, len(windows), qw_groups):
        ws = windows[g0 : g0 + qw_groups]
        sets = [
            {(sid, nblk) for sid, nblk, valid in w if valid.all()}
            for w, _g in ws
        ]
        shared = sorted(set.intersection(*sets)) if len(ws) == qw_groups else []
        shared_set = set(shared)
        owns = []
        for w, guard in ws:
            owns.append(
                (
                    [
                        (sid, nblk, valid)
                        for sid, nblk, valid in w
                        if (sid, nblk) not in shared_set or not valid.all()
                    ],
                    guard,
                )
            )
        groups.append((shared, owns))
    return groups


def _mask_table(windows, qw_groups=4):
    """Dedup mask patterns over own-chunk pairs. Returns (n_masks, map from
    pair-signature -> mask id); pairs with no zero region map to None."""
    mk_of_pair = {}
    n = 0
    groups = _group_plan(windows, qw_groups)
    for shared, owns in groups:
        for own, _guard in owns:
            for i0 in range(0, len(own), 2):
                pair = own[i0 : i0 + 2]
                sig = tuple(
                    (sid, nblk, valid.tobytes()) for sid, nblk, valid in pair
                )
                if sig in mk_of_pair:
                    continue
                has = any(
                    not valid.all() for _sid, _nblk, valid in pair
                )
                if has:
                    mk_of_pair[sig] = n
                    n += 1
    return n, mk_of_pair


def _build_masks(windows, dims, p_np, qw_groups=4):
    """Materialize the deduplicated mask tiles: [n_mk, 128, 2, nq]."""
    S, L, HD, bs, qw, nch = dims
    nq = qw * bs
    n_mk, mk_of_pair = _mask_table(windows, qw_groups)
    mk = np.ones((max(n_mk, 1), 128, 2, nq), p_np)
    groups = _group_plan(windows, qw_groups)
    done = set()
    for shared, owns in groups:
        for own, _guard in owns:
            for i0 in range(0, len(own), 2):
                pair = own[i0 : i0 + 2]
                sig = tuple(
                    (sid, nblk, valid.tobytes()) for sid, nblk, valid in pair
                )
                mid = mk_of_pair.get(sig)
                if mid is None or mid in done:
                    continue
                done.add(mid)
                for h, (_sid, _nblk, valid) in enumerate(pair):
                    for p0, p1, c0, c1 in _zero_regions(valid, bs):
                        mk[mid, p0:p1, h, c0:c1] = 0
    return mk


def _build_nc(windows, slot_blocks, dims, s_dt_name, p_dt_name, repeat):
    import hashlib
    import os

    import concourse.bass as bass
    import concourse.mybir as mybir
    import concourse.tile as tile
    from concourse import bacc

    ablate = set(os.environ.get("KERNEL_ABLATE", "").split(","))
    stp_bufs = int(os.environ.get("TUNE_STP", "4"))
    ptp_bufs = int(os.environ.get("TUNE_PTP", "14"))
    ovp_bufs = int(os.environ.get("TUNE_OVP", "2"))
    mark_reps = os.environ.get("MARK_REPS", "") == "1"

    # config fingerprint -> rtag length, so no two program variants share an
    # input signature (the neuron compile cache can alias same-signature HLO)
    cfg = repr((_KVER, sorted(ablate), stp_bufs, ptp_bufs, ovp_bufs,
                mark_reps, s_dt_name, p_dt_name, repeat, dims,
                [tuple(b) for b in slot_blocks]))
    cfg_h = int(hashlib.sha256(cfg.encode()).hexdigest(), 16) % 769

    S, L, HD, bs, qw, nch = dims
    s_dt = getattr(mybir.dt, s_dt_name)
    p_dt = getattr(mybir.dt, p_dt_name)
    f32 = mybir.dt.float32
    nq = qw * bs
    scale = float(HD) ** -0.5

    nwin = L // nq
    nc = bacc.Bacc("TRN2", debug=False)
    # dummy repeat-sized input: makes each repeat-variant's HLO structurally
    # unique so the neuron compile cache cannot alias them
    rtag_len = 16 * repeat + cfg_h
    rtag_d = nc.dram_tensor("rtag", [1, rtag_len], mybir.dt.float32,
                            kind="ExternalInput")
    qt_d = nc.dram_tensor("qt", [S, HD, L], s_dt, kind="ExternalInput")
    ktg_d = nc.dram_tensor("ktg", [S, HD, nch * 128], s_dt, kind="ExternalInput")
    vg_d = nc.dram_tensor("vg", [S, 128, nch, HD + 1], p_dt, kind="ExternalInput")
    # deduplicated 0/1 mask tiles for own-chunk pairs (possibly zero patterns)
    n_mk, mk_of_pair = _mask_table(windows)
    mk_d = nc.dram_tensor("mk", [max(n_mk, 1), 128, 2, nq], p_dt,
                          kind="ExternalInput")
    # p-major output: out[s, p, w, d] = O[s, w*nq + p, d]; host untransposes
    out_d = nc.dram_tensor("out", [S, nq, nwin, HD], f32, kind="ExternalOutput")
    groups = _group_plan(windows)

    with tile.TileContext(nc) as tc:
        with (
            tc.tile_pool(name="big", bufs=2) as big,
            tc.tile_pool(name="ptp", bufs=ptp_bufs) as ptp,
            tc.tile_pool(name="onp", bufs=4) as onp,
            tc.tile_pool(name="stp", bufs=stp_bufs, space="PSUM") as stp,
            tc.tile_pool(name="ovp", bufs=ovp_bufs, space="PSUM") as ovp,
        ):
            rtag_t = big.tile([1, rtag_len], mybir.dt.float32, tag="rtag")
            nc.sync.dma_start(out=rtag_t, in_=rtag_d.ap())
            mk_t = big.tile([128, max(n_mk, 1), 2, nq], p_dt, tag="mk", bufs=1)
            nc.sync.dma_start(
                out=mk_t, in_=mk_d.ap().rearrange("n p h q -> p n h q")
            )
            for _rep in range(repeat):
                for s in range(S):
                    qt_t = big.tile([HD, L], s_dt, tag="qt")
                    nc.sync.dma_start(out=qt_t, in_=qt_d.ap()[s])
                    ktg_t = big.tile([HD, nch * 128], s_dt, tag="ktg")
                    nc.sync.dma_start(out=ktg_t, in_=ktg_d.ap()[s])
                    vg_t = big.tile([128, nch, HD + 1], p_dt, tag="vg")
                    nc.sync.dma_start(out=vg_t, in_=vg_d.ap()[s])
                    o_slice = big.tile([nq, nwin, HD], f32, tag="o_slice")
                    for gi, (shared, owns) in enumerate(groups):
                        gw = len(owns)            # windows in this group
                        gq = gw * nq              # group query width
                        q0 = gi * 4 * nq          # group query start
                        # one PSUM bank holds all gw window accumulators
                        ov_t = ovp.tile([128, gw, HD + 1], f32, tag="ov")
                        # group-shared chunks: S_T+exp once at width gq
                        sh_pts = []
                        for sid, nblk in shared:
                            nk = nblk * bs
                            st_t = stp.tile([128, gq], f32, tag="st")
                            if "skipst" not in ablate:
                                nc.tensor.matmul(
                                    st_t[:nk, :],
                                    lhsT=ktg_t[:, sid * 128 : sid * 128 + nk],
                                    rhs=qt_t[:, q0 : q0 + gq],
                                    start=True,
                                    stop=True,
                                )
                            pt_t = ptp.tile([128, gq], p_dt, tag="pt")
                            if "skipexp" not in ablate:
                                nc.scalar.activation(
                                    pt_t[:nk, :],
                                    st_t[:nk, :],
                                    mybir.ActivationFunctionType.Exp,
                                    scale=scale,
                                )
                            sh_pts.append((pt_t, sid, nk))
                        # per-window extra chunks (masked/partial): pack up
                        # to two chunks into one PSUM tile / one exp call
                        own_pts = []
                        for m, (own, _guard) in enumerate(owns):
                            wpts = []
                            for i0 in range(0, len(own), 2):
                                pair = own[i0 : i0 + 2]
                                np_ = len(pair)
                                st_t = stp.tile([128, 2, nq], f32, tag="st")
                                pt_t = ptp.tile([128, 2, nq], p_dt, tag="pt")
                                for h, (sid, nblk, valid) in enumerate(pair):
                                    nk = nblk * bs
                                    if "skipst" not in ablate:
                                        nc.tensor.matmul(
                                            st_t[:nk, h, :],
                                            lhsT=ktg_t[
                                                :, sid * 128 : sid * 128 + nk
                                            ],
                                            rhs=qt_t[
                                                :,
                                                q0 + m * nq : q0 + (m + 1) * nq,
                                            ],
                                            start=True,
                                            stop=True,
                                        )
                                if "skipexp" not in ablate:
                                    # full-tile exp; stale rows beyond each
                                    # chunk's nk are never read downstream
                                    nc.scalar.activation(
                                        pt_t[:, 0:np_, :],
                                        st_t[:, 0:np_, :],
                                        mybir.ActivationFunctionType.Exp,
                                        scale=scale,
                                    )
                                mid = mk_of_pair.get(
                                    tuple(
                                        (sid, nblk, valid.tobytes())
                                        for sid, nblk, valid in pair
                                    )
                                )
                                if mid is not None and "nomask" not in ablate:
                                    nc.vector.tensor_mul(
                                        pt_t[:, 0:np_, :],
                                        pt_t[:, 0:np_, :],
                                        mk_t[:, mid, 0:np_, :],
                                    )
                                for h, (sid, nblk, valid) in enumerate(pair):
                                    wpts.append((pt_t, sid, nblk * bs, h))
                            own_pts.append(wpts)
                        # PV accumulation per window
                        if "nopv" not in ablate:
                            for m, wpts in enumerate(own_pts):
                                npv = len(sh_pts) + len(wpts)
                                ci = 0
                                for pt_t, sid, nk in sh_pts:
                                    nc.tensor.matmul(
                                        ov_t[:, m, :],
                                        lhsT=pt_t[:nk, m * nq : (m + 1) * nq],
                                        rhs=vg_t[:nk, sid, :],
                                        start=(ci == 0),
                                        stop=(ci == npv - 1),
                                    )
                                    ci += 1
                                for pt_t, sid, nk, h in wpts:
                                    nc.tensor.matmul(
                                        ov_t[:, m, :],
                                        lhsT=pt_t[:nk, h, :],
                                        rhs=vg_t[:nk, sid, :],
                                        start=(ci == 0),
                                        stop=(ci == npv - 1),
                                    )
                                    ci += 1
                        # normalize into the slice-wide output tile:
                        # one strided reciprocal + one broadcast multiply
                        # covers all gw windows of the group
                        if "nopv" in ablate:
                            nc.vector.memset(
                                o_slice[:, gi * 4 : gi * 4 + gw, :], 0.0
                            )
                        else:
                            rec_t = onp.tile([128, gw], f32, tag="rec")
                            if any(g for _o, g in owns):
                                den_t = onp.tile([128, gw], f32, tag="den")
                                nc.vector.tensor_scalar_max(
                                    den_t, ov_t[:, :, HD], 1e-37
                                )
                                nc.vector.reciprocal(rec_t, den_t)
                            else:
                                nc.vector.reciprocal(rec_t, ov_t[:, :, HD])
                            rec_b = bass.AP(
                                tensor=rec_t.tensor,
                                offset=rec_t.offset,
                                ap=list(rec_t.ap) + [[0, HD]],
                            )
                            nc.vector.tensor_mul(
                                o_slice[:, gi * 4 : gi * 4 + gw, :],
                                ov_t[:, :, 0:HD],
                                rec_b,
                            )
                        if mark_reps:
                            nc.scalar.mul(
                                o_slice[:, gi * 4 : gi * 4 + gw, :],
                                o_slice[:, gi * 4 : gi * 4 + gw, :],
                                float(_rep + 1),
                            )
                    nc.sync.dma_start(out=out_d.ap()[s], in_=o_slice)
    nc.compile()
    return nc


def _np_dt(name):
    if name == "float32":
        return np.float32
    if name == "float16":
        return np.float16
    import ml_dtypes

    return np.dtype(getattr(ml_dtypes, name))


def _prepare(query, key, value, rows, cols, block, repeat):
    B, H, L, HD = query.shape
    bs = int(block)
    nb = L // bs
    qw = max(1, 128 // bs)
    cache_key = (
        query.shape,
        bs,
        rows.tobytes(),
        cols.tobytes(),
        S_DT_NAME,
        P_DT_NAME,
        repeat,
    )
    if cache_key in _cache:
        return _cache[cache_key]

    windows, slot_blocks = _build_plan(np.asarray(rows), np.asarray(cols), nb, qw)
    nch = len(slot_blocks)
    dims = (B * H // N_CORES, L, HD, bs, qw, nch)
    nc = _build_nc(windows, slot_blocks, dims, S_DT_NAME, P_DT_NAME, repeat)
    _cache[cache_key] = (nc, windows, slot_blocks, dims)
    return _cache[cache_key]


def kernel(query, key, value, rows, cols, block):
    from concourse import bass_utils

    query = np.asarray(query)
    key = np.asarray(key)
    value = np.asarray(value)
    rows = np.asarray(rows)
    cols = np.asarray(cols)

    nc, windows, slot_blocks, dims = _prepare(
        query, key, value, rows, cols, block, repeat=1
    )
    S, L, HD, bs, qw, nch = dims
    B, H = query.shape[0], query.shape[1]
    BH = B * H
    s_np = _np_dt(S_DT_NAME)
    p_np = _np_dt(P_DT_NAME)

    q2 = query.reshape(BH, L, HD)
    k2 = key.reshape(BH, L, HD)
    v2 = value.reshape(BH, L, HD)
    qt = np.ascontiguousarray(q2.transpose(0, 2, 1)).astype(s_np)
    ktg = np.zeros((BH, HD, nch, 128), s_np)
    vg = np.zeros((BH, 128, nch, HD + 1), p_np)
    for sid, blocks in enumerate(slot_blocks):
        for kb, c in enumerate(blocks):
            kblk = k2[:, c * bs : (c + 1) * bs, :]
            ktg[:, :, sid, kb * bs : (kb + 1) * bs] = kblk.transpose(0, 2, 1)
            vg[:, kb * bs : (kb + 1) * bs, sid, :HD] = v2[:, c * bs : (c + 1) * bs, :]
            vg[:, kb * bs : (kb + 1) * bs, sid, HD] = 1.0
    ktg = ktg.reshape(BH, HD, nch * 128)

    rtag_len = None
    for alloc in nc.m.functions[0].allocations:
        if getattr(alloc, "memorylocations", None) and \
                alloc.memorylocations[0].name == "rtag":
            rtag_len = alloc.tensor_shape[1]
    rtag = np.zeros((1, rtag_len), np.float32)
    mk = _build_masks(windows, dims, p_np)
    in_maps = []
    for c in range(N_CORES):
        sl = slice(c * S, (c + 1) * S)
        in_maps.append({"qt": qt[sl], "ktg": ktg[sl], "vg": vg[sl],
                        "rtag": rtag, "mk": mk})

    res = bass_utils.run_bass_kernel_spmd(nc, in_maps, core_ids=list(range(N_CORES)))
    out = np.stack([res.results[c]["out"] for c in range(N_CORES)])
    # out: [8, S, nq, nwin, HD] p-major -> [BH, L, HD]
    nq = out.shape[2]
    out = out.reshape(BH, nq, L // nq, HD).transpose(0, 2, 1, 3)
    return out.reshape(B, H, L, HD).astype(np.float32)


# revision 20
# speedup vs baseline: 1.3642x; 1.0059x over previous
"""Block-sparse (DeepSpeed fixed-layout) self-attention on 8 Trainium2 cores.

Strategy
--------
Shard the 32 (batch, head) slices across 8 cores (4 each, pure data parallel).
For each slice, queries are processed in windows of 128 rows (4 key-blocks of
32). The union of active key blocks for a window is split into "chunk slots"
of up to 128 keys; slots are deduplicated across windows (the fixed layout's
global stripe makes most slots shared). Host-side numpy pre-gathers:
  QT  [S, 64, L]            query transposed (hd on partitions)
  KTg [S, 64, nch*128]      gathered+transposed key chunks
  Vg  [S, 128, nch, 65]     gathered value chunks with a ones column
On device, per window and chunk:
  S_T  = KT_chunk.T-matmul (scores arrive keys-on-partitions: no P transpose)
  P    = exp(scale * S_T)   on ACT, straight from PSUM
  mask: memset invalid (key-block, query-block) cells to zero
  O~  += P.T @ [V | 1]      accumulated in PSUM; col 64 = softmax denominator
then O = O~[:, :64] * (1 / O~[:, 64]) and DMA out. exp() needs no max
subtraction: scores are ~N(0,1) after scaling, far from fp32 overflow.
"""

import sys

sys.path.insert(0, "/opt/trn_rl_repo")

import numpy as np

N_CORES = 8
_KVER = "v9"  # bump on any codegen change: feeds the rtag config fingerprint

# dtype knobs: storage+matmul dtype for scores (QT/KTg) and probs (P/Vg)
S_DT_NAME = "float16"
P_DT_NAME = "float16"

_cache = {}


def _build_plan(rows, cols, nb, qw):
    """Per query-window chunk lists + deduplicated key-chunk slots.

    Returns (windows, slot_blocks):
      windows: list (one per window) of chunks (slot_id, n_blocks, valid)
               where valid[kb, j] says whether key-block kb of the chunk is
               attended by query-block j of the window.
      slot_blocks: slot_id -> list of key block ids stored in that slot.
    """
    from collections import Counter

    row_cnt = [Counter() for _ in range(nb)]
    for r, c in zip(rows.tolist(), cols.tolist()):
        row_cnt[int(r)][int(c)] += 1

    slots = {}
    slot_blocks = []
    windows = []
    for w0 in range(0, nb, qw):
        cnts = [row_cnt[w0 + j] for j in range(qw)]
        cols_set = sorted(set().union(*[set(c.keys()) for c in cnts]))
        entries = []  # (block, occurrence)
        for c in cols_set:
            m = max(cnt[c] for cnt in cnts)
            entries.extend((c, k) for k in range(m))
        # maximal equal-stride runs -> chunk boundaries shared across windows
        runs = []
        i, n = 0, len(entries)
        while i < n:
            if i + 1 < n:
                stride = entries[i + 1][0] - entries[i][0]
                j = i + 1
                while j + 1 < n and entries[j + 1][0] - entries[j][0] == stride:
                    j += 1
            else:
                stride, j = 1, i
            runs.append((i, j + 1, stride))
            i = j + 1
        wchunks = []
        for a, b, stride in runs:
            for t in range(a, b, qw):
                grp = entries[t : min(t + qw, b)]
                start = grp[0][0]
                key = (start, stride if len(grp) > 1 else 1)
                sid = slots.get(key)
                if sid is None:
                    sid = len(slot_blocks)
                    slots[key] = sid
                    slot_blocks.append([])
                if len(slot_blocks[sid]) < len(grp):
                    slot_blocks[sid] = [start + key[1] * u for u in range(len(grp))]
                valid = np.ones((len(grp), qw), dtype=bool)
                for kb, (c, k) in enumerate(grp):
                    for j in range(qw):
                        valid[kb, j] = k < cnts[j][c]
                wchunks.append((sid, len(grp), valid))
        # does any query row of this window have no valid key at all?
        anyvalid = np.zeros(qw, dtype=bool)
        for _sid, _n, valid in wchunks:
            anyvalid |= valid.any(axis=0)
        windows.append((wchunks, not anyvalid.all()))
    return windows, slot_blocks


def _zero_regions(valid, bs):
    """Invalid (key-block, query-block) cells as memset rectangles."""
    regs = []
    nkb, qw = valid.shape
    for kb in range(nkb):
        j = 0
        while j < qw:
            if not valid[kb, j]:
                j0 = j
                while j < qw and not valid[kb, j]:
                    j += 1
                regs.append((kb * bs, (kb + 1) * bs, j0 * bs, j * bs))
            else:
                j += 1
    return regs


def _group_plan(windows, qw_groups=4):
    """Group consecutive windows; extract chunks shared (all-valid) by every
    window of the group so their S_T/exp run once at group width."""
    groups = []
    for g0 in range(0, len(windows), qw_groups):
        ws = windows[g0 : g0 + qw_groups]
        sets = [
            {(sid, nblk) for sid, nblk, valid in w if valid.all()}
            for w, _g in ws
        ]
        shared = sorted(set.intersection(*sets)) if len(ws) == qw_groups else []
        shared_set = set(shared)
        owns = []
        for w, guard in ws:
            owns.append(
                (
                    [
                        (sid, nblk, valid)
                        for sid, nblk, valid in w
                        if (sid, nblk) not in shared_set or not valid.all()
                    ],
                    guard,
                )
            )
        groups.append((shared, owns))
    return groups


def _quads(owns):
    """Flatten a group's per-window own chunks and pack 4 halves per tile."""
    halves = []
    for m, (own, _guard) in enumerate(owns):
        for sid, nblk, valid in own:
            halves.append((m, sid, nblk, valid))
    return [halves[i : i + 4] for i in range(0, len(halves), 4)]


def _quad_sig(quad):
    # mask pattern depends only on each half's (nblk, validity) - not sid
    return tuple((nblk, valid.tobytes()) for _m, _sid, nblk, valid in quad)


def _mask_table(windows, qw_groups=4):
    """Dedup mask patterns over own-chunk quads. Returns (n_masks, map from
    quad-signature -> mask id); quads with no zero region map to nothing."""
    mk_of = {}
    n = 0
    for shared, owns in _group_plan(windows, qw_groups):
        for quad in _quads(owns):
            sig = _quad_sig(quad)
            if sig in mk_of:
                continue
            if any(not valid.all() for _m, _sid, _nblk, valid in quad):
                mk_of[sig] = n
                n += 1
    return n, mk_of


def _build_masks(windows, dims, p_np, qw_groups=4):
    """Materialize the deduplicated mask tiles: [n_mk, 128, 4, nq]."""
    S, L, HD, bs, qw, nch = dims
    nq = qw * bs
    n_mk, mk_of = _mask_table(windows, qw_groups)
    mk = np.ones((max(n_mk, 1), 128, 4, nq), p_np)
    done = set()
    for shared, owns in _group_plan(windows, qw_groups):
        for quad in _quads(owns):
            mid = mk_of.get(_quad_sig(quad))
            if mid is None or mid in done:
                continue
            done.add(mid)
            for h, (_m, _sid, _nblk, valid) in enumerate(quad):
                for p0, p1, c0, c1 in _zero_regions(valid, bs):
                    mk[mid, p0:p1, h, c0:c1] = 0
    return mk


def _build_nc(windows, slot_blocks, dims, s_dt_name, p_dt_name, repeat):
    import hashlib
    import os

    import concourse.bass as bass
    import concourse.mybir as mybir
    import concourse.tile as tile
    from concourse import bacc

    ablate = set(os.environ.get("KERNEL_ABLATE", "").split(","))
    stp_bufs = int(os.environ.get("TUNE_STP", "4"))
    ptp_bufs = int(os.environ.get("TUNE_PTP", "14"))
    ovp_bufs = int(os.environ.get("TUNE_OVP", "2"))
    mark_reps = os.environ.get("MARK_REPS", "") == "1"

    # config fingerprint -> rtag length, so no two program variants share an
    # input signature (the neuron compile cache can alias same-signature HLO)
    cfg = repr((_KVER, sorted(ablate), stp_bufs, ptp_bufs, ovp_bufs,
                mark_reps, s_dt_name, p_dt_name, repeat, dims,
                [tuple(b) for b in slot_blocks]))
    cfg_h = int(hashlib.sha256(cfg.encode()).hexdigest(), 16) % 769

    S, L, HD, bs, qw, nch = dims
    s_dt = getattr(mybir.dt, s_dt_name)
    p_dt = getattr(mybir.dt, p_dt_name)
    f32 = mybir.dt.float32
    nq = qw * bs
    scale = float(HD) ** -0.5

    nwin = L // nq
    nc = bacc.Bacc("TRN2", debug=False)
    # dummy repeat-sized input: makes each repeat-variant's HLO structurally
    # unique so the neuron compile cache cannot alias them
    rtag_len = 16 * repeat + cfg_h
    rtag_d = nc.dram_tensor("rtag", [1, rtag_len], mybir.dt.float32,
                            kind="ExternalInput")
    qt_d = nc.dram_tensor("qt", [S, HD, L], s_dt, kind="ExternalInput")
    ktg_d = nc.dram_tensor("ktg", [S, HD, nch * 128], s_dt, kind="ExternalInput")
    vg_d = nc.dram_tensor("vg", [S, 128, nch, HD + 1], p_dt, kind="ExternalInput")
    # deduplicated 0/1 mask tiles for own-chunk pairs (possibly zero patterns)
    n_mk, mk_of = _mask_table(windows)
    mk_d = nc.dram_tensor("mk", [max(n_mk, 1), 128, 4, nq], p_dt,
                          kind="ExternalInput")
    # p-major output: out[s, p, w, d] = O[s, w*nq + p, d]; host untransposes
    out_d = nc.dram_tensor("out", [S, nq, nwin, HD], f32, kind="ExternalOutput")
    groups = _group_plan(windows)

    with tile.TileContext(nc) as tc:
        with (
            tc.tile_pool(name="big", bufs=2) as big,
            tc.tile_pool(name="ptp", bufs=ptp_bufs) as ptp,
            tc.tile_pool(name="onp", bufs=4) as onp,
            tc.tile_pool(name="stp", bufs=stp_bufs, space="PSUM") as stp,
            tc.tile_pool(name="ovp", bufs=ovp_bufs, space="PSUM") as ovp,
        ):
            rtag_t = big.tile([1, rtag_len], mybir.dt.float32, tag="rtag")
            nc.sync.dma_start(out=rtag_t, in_=rtag_d.ap())
            mk_t = big.tile([128, max(n_mk, 1), 4, nq], p_dt, tag="mk", bufs=1)
            nc.sync.dma_start(
                out=mk_t, in_=mk_d.ap().rearrange("n p h q -> p n h q")
            )
            for _rep in range(repeat):
                for s in range(S):
                    qt_t = big.tile([HD, L], s_dt, tag="qt")
                    nc.sync.dma_start(out=qt_t, in_=qt_d.ap()[s])
                    ktg_t = big.tile([HD, nch * 128], s_dt, tag="ktg")
                    nc.sync.dma_start(out=ktg_t, in_=ktg_d.ap()[s])
                    vg_t = big.tile([128, nch, HD + 1], p_dt, tag="vg")
                    nc.sync.dma_start(out=vg_t, in_=vg_d.ap()[s])
                    o_slice = big.tile([nq, nwin, HD], f32, tag="o_slice")
                    for gi, (shared, owns) in enumerate(groups):
                        gw = len(owns)            # windows in this group
                        gq = gw * nq              # group query width
                        q0 = gi * 4 * nq          # group query start
                        # one PSUM bank holds all gw window accumulators
                        ov_t = ovp.tile([128, gw, HD + 1], f32, tag="ov")
                        # group-shared chunks: S_T+exp once at width gq
                        sh_pts = []
                        for sid, nblk in shared:
                            nk = nblk * bs
                            st_t = stp.tile([128, gq], f32, tag="st")
                            if "skipst" not in ablate:
                                nc.tensor.matmul(
                                    st_t[:nk, :],
                                    lhsT=ktg_t[:, sid * 128 : sid * 128 + nk],
                                    rhs=qt_t[:, q0 : q0 + gq],
                                    start=True,
                                    stop=True,
                                )
                            pt_t = ptp.tile([128, gq], p_dt, tag="pt")
                            if "skipexp" not in ablate:
                                nc.scalar.activation(
                                    pt_t[:nk, :],
                                    st_t[:nk, :],
                                    mybir.ActivationFunctionType.Exp,
                                    scale=scale,
                                )
                            sh_pts.append((pt_t, sid, nk))
                        # per-window extra chunks (masked/partial):
                        # pack 4 halves (across windows) per PSUM bank,
                        # one exp + one mask-mul per quad
                        own_pts = [[] for _ in owns]
                        for quad in _quads(owns):
                            nh = len(quad)
                            st_t = stp.tile([128, 4, nq], f32, tag="st")
                            pt_t = ptp.tile([128, 4, nq], p_dt, tag="pt")
                            for h, (m, sid, nblk, valid) in enumerate(quad):
                                nk = nblk * bs
                                if "skipst" not in ablate:
                                    nc.tensor.matmul(
                                        st_t[:nk, h, :],
                                        lhsT=ktg_t[
                                            :, sid * 128 : sid * 128 + nk
                                        ],
                                        rhs=qt_t[
                                            :,
                                            q0 + m * nq : q0 + (m + 1) * nq,
                                        ],
                                        start=True,
                                        stop=True,
                                    )
                            if "skipexp" not in ablate:
                                # full-tile exp; stale rows beyond each
                                # chunk's nk are never read downstream
                                nc.scalar.activation(
                                    pt_t[:, 0:nh, :],
                                    st_t[:, 0:nh, :],
                                    mybir.ActivationFunctionType.Exp,
                                    scale=scale,
                                )
                            mid = mk_of.get(_quad_sig(quad))
                            if mid is not None and "nomask" not in ablate:
                                nc.vector.tensor_mul(
                                    pt_t[:, 0:nh, :],
                                    pt_t[:, 0:nh, :],
                                    mk_t[:, mid, 0:nh, :],
                                )
                            for h, (m, sid, nblk, valid) in enumerate(quad):
                                own_pts[m].append((pt_t, sid, nblk * bs, h))
                        # PV accumulation per window
                        if "nopv" not in ablate:
                            for m, wpts in enumerate(own_pts):
                                npv = len(sh_pts) + len(wpts)
                                ci = 0
                                for pt_t, sid, nk in sh_pts:
                                    nc.tensor.matmul(
                                        ov_t[:, m, :],
                                        lhsT=pt_t[:nk, m * nq : (m + 1) * nq],
                                        rhs=vg_t[:nk, sid, :],
                                        start=(ci == 0),
                                        stop=(ci == npv - 1),
                                    )
                                    ci += 1
                                for pt_t, sid, nk, h in wpts:
                                    nc.tensor.matmul(
                                        ov_t[:, m, :],
                                        lhsT=pt_t[:nk, h, :],
                                        rhs=vg_t[:nk, sid, :],
                                        start=(ci == 0),
                                        stop=(ci == npv - 1),
                                    )
                                    ci += 1
                        # normalize into the slice-wide output tile:
                        # one strided reciprocal + one broadcast multiply
                        # covers all gw windows of the group
                        if "nopv" in ablate:
                            nc.vector.memset(
                                o_slice[:, gi * 4 : gi * 4 + gw, :], 0.0
                            )
                        else:
                            rec_t = onp.tile([128, gw], f32, tag="rec")
                            if any(g for _o, g in owns):
                                den_t = onp.tile([128, gw], f32, tag="den")
                                nc.vector.tensor_scalar_max(
                                    den_t, ov_t[:, :, HD], 1e-37
                                )
                                nc.vector.reciprocal(rec_t, den_t)
                            else:
                                nc.vector.reciprocal(rec_t, ov_t[:, :, HD])
                            rec_b = bass.AP(
                                tensor=rec_t.tensor,
                                offset=rec_t.offset,
                                ap=list(rec_t.ap) + [[0, HD]],
                            )
                            nc.vector.tensor_mul(
                                o_slice[:, gi * 4 : gi * 4 + gw, :],
                                ov_t[:, :, 0:HD],
                                rec_b,
                            )
                        if mark_reps:
                            nc.scalar.mul(
                                o_slice[:, gi * 4 : gi * 4 + gw, :],
                                o_slice[:, gi * 4 : gi * 4 + gw, :],
                                float(_rep + 1),
                            )
                    nc.sync.dma_start(out=out_d.ap()[s], in_=o_slice)
    nc.compile()
    return nc


def _np_dt(name):
    if name == "float32":
        return np.float32
    if name == "float16":
        return np.float16
    import ml_dtypes

    return np.dtype(getattr(ml_dtypes, name))


def _prepare(query, key, value, rows, cols, block, repeat):
    B, H, L, HD = query.shape
    bs = int(block)
    nb = L // bs
    qw = max(1, 128 // bs)
    cache_key = (
        query.shape,
        bs,
        rows.tobytes(),
        cols.tobytes(),
        S_DT_NAME,
        P_DT_NAME,
        repeat,
    )
    if cache_key in _cache:
        return _cache[cache_key]

    windows, slot_blocks = _build_plan(np.asarray(rows), np.asarray(cols), nb, qw)
    nch = len(slot_blocks)
    dims = (B * H // N_CORES, L, HD, bs, qw, nch)
    nc = _build_nc(windows, slot_blocks, dims, S_DT_NAME, P_DT_NAME, repeat)
    _cache[cache_key] = (nc, windows, slot_blocks, dims)
    return _cache[cache_key]


def kernel(query, key, value, rows, cols, block):
    from concourse import bass_utils

    query = np.asarray(query)
    key = np.asarray(key)
    value = np.asarray(value)
    rows = np.asarray(rows)
    cols = np.asarray(cols)

    nc, windows, slot_blocks, dims = _prepare(
        query, key, value, rows, cols, block, repeat=1
    )
    S, L, HD, bs, qw, nch = dims
    B, H = query.shape[0], query.shape[1]
    BH = B * H
    s_np = _np_dt(S_DT_NAME)
    p_np = _np_dt(P_DT_NAME)

    q2 = query.reshape(BH, L, HD)
    k2 = key.reshape(BH, L, HD)
    v2 = value.reshape(BH, L, HD)
    qt = np.ascontiguousarray(q2.transpose(0, 2, 1)).astype(s_np)
    ktg = np.zeros((BH, HD, nch, 128), s_np)
    vg = np.zeros((BH, 128, nch, HD + 1), p_np)
    for sid, blocks in enumerate(slot_blocks):
        for kb, c in enumerate(blocks):
            kblk = k2[:, c * bs : (c + 1) * bs, :]
            ktg[:, :, sid, kb * bs : (kb + 1) * bs] = kblk.transpose(0, 2, 1)
            vg[:, kb * bs : (kb + 1) * bs, sid, :HD] = v2[:, c * bs : (c + 1) * bs, :]
            vg[:, kb * bs : (kb + 1) * bs, sid, HD] = 1.0
    ktg = ktg.reshape(BH, HD, nch * 128)

    rtag_len = None
    for alloc in nc.m.functions[0].allocations:
        if getattr(alloc, "memorylocations", None) and \
                alloc.memorylocations[0].name == "rtag":
            rtag_len = alloc.tensor_shape[1]
    rtag = np.zeros((1, rtag_len), np.float32)
    mk = _build_masks(windows, dims, p_np)
    in_maps = []
    for c in range(N_CORES):
        sl = slice(c * S, (c + 1) * S)
        in_maps.append({"qt": qt[sl], "ktg": ktg[sl], "vg": vg[sl],
                        "rtag": rtag, "mk": mk})

    res = bass_utils.run_bass_kernel_spmd(nc, in_maps, core_ids=list(range(N_CORES)))
    out = np.stack([res.results[c]["out"] for c in range(N_CORES)])
    # out: [8, S, nq, nwin, HD] p-major -> [BH, L, HD]
    nq = out.shape[2]
    out = out.reshape(BH, nq, L // nq, HD).transpose(0, 2, 1, 3)
    return out.reshape(B, H, L, HD).astype(np.float32)
